# revision 18
# baseline (speedup 1.0000x reference)
# Trainium2 Bass kernel for nn_DeepST_model_79422535237670 (DeepST/SGFormer GNN).
# Strategy: data-parallel over nodes on 8 cores. Encoder via bf16 DMA-transpose
# matmuls; linear attention with tiny AllReduce of [kvs|ks_sum|ssq]; GCN
# aggregation via AllGather of h tables + dma_gather of edge rows + weighted
# one-hot matmul scatter (host-prebuilt bf16 one-hot, PSUM accumulate).
import numpy as np
import ml_dtypes

import concourse.bacc as bacc
import concourse.bass as bass
import concourse.tile as tile
import concourse.mybir as mybir
from concourse.bass_utils import run_bass_kernel_spmd

F32 = mybir.dt.float32
BF16 = mybir.dt.bfloat16
I16 = mybir.dt.int16
I32 = mybir.dt.int32
AX = mybir.AxisListType
OP = mybir.AluOpType
ACT = mybir.ActivationFunctionType

N, D_IN, E_EDGES = 50000, 3000, 300000
NCORE = 8
NPC = N // NCORE          # 6250
NPAD = 6272               # 49*128
CH = NPAD // 128          # 49 node chunks per core
KPAD = 3072               # 24*128 padded D_IN
KC = KPAD // 128          # 24
TBL = NPAD * NCORE        # 50176 table rows (AllGather layout)
SPLIT = TBL // 2          # 25088: pass A rows [0,SPLIT), pass B [SPLIT,TBL)
HID = 64
NODEGROUPS = [(0, 2048), (2048, 2048), (4096, 2048), (6144, 128)]
CGRP = 4                  # chunks per gather segment
BN_EPS = 1e-3
BN1D_EPS = 1e-5
LN_EPS = 1e-5
ALPHA_DEC = 0.9
ALPHA_TRANS = 0.5
GRAPH_W = 0.8

_CACHE = {}


# --------------------------------------------------------------------------
# host preprocessing
# --------------------------------------------------------------------------
def _np(v):
    return np.asarray(v)


def _build_plan(edge_index):
    src = _np(edge_index[0]).astype(np.int64)
    dst = _np(edge_index[1]).astype(np.int64)
    deg = 1.0 + np.bincount(dst, minlength=N).astype(np.float64)
    dinv = 1.0 / np.sqrt(deg)
    s_all = np.concatenate([src, np.arange(N)])
    d_all = np.concatenate([dst, np.arange(N)])
    w_all = (dinv[s_all] * dinv[d_all]).astype(np.float32)

    core_of = d_all // NPC
    local_dst = d_all - core_of * NPC
    chunk = local_dst // 128
    tbl_idx = (s_all // NPC) * NPAD + (s_all % NPC)
    pas = (tbl_idx >= SPLIT).astype(np.int64)

    cnt = np.zeros((NCORE, CH, 2), np.int64)
    np.add.at(cnt, (core_of, chunk, pas), 1)
    tiles = np.maximum(1, (cnt + 127) // 128).max(axis=0)  # [CH,2]
    toff = np.zeros((2, CH), np.int64)
    t = 0
    for p in range(2):
        for k in range(CH):
            toff[p, k] = t
            t += tiles[k, p]
    TT = int(t)
    S = TT * 128

    idx16 = np.zeros((NCORE, S), np.int16)
    d_loc = np.zeros((NCORE, S), np.int32)
    w_slot = np.zeros((NCORE, S), np.float32)
    key = core_of * (CH * 2) + chunk * 2 + pas
    order = np.argsort(key, kind="stable")
    ko = key[order]
    grp_start = np.searchsorted(ko, np.arange(NCORE * CH * 2))
    grp_end = np.searchsorted(ko, np.arange(NCORE * CH * 2), side="right")
    for c in range(NCORE):
        for k in range(CH):
            for p in range(2):
                gi = c * (CH * 2) + k * 2 + p
                es = order[grp_start[gi]:grp_end[gi]]
                n = len(es)
                if n == 0:
                    continue
                s0 = int(toff[p, k] * 128)
                idx16[c, s0:s0 + n] = (tbl_idx[es] - p * SPLIT).astype(np.int16)
                d_loc[c, s0:s0 + n] = (local_dst[es] - k * 128).astype(np.int32)
                w_slot[c, s0:s0 + n] = w_all[es]

    sarange = np.arange(S)
    oh = np.zeros((NCORE, 128, S), np.float32)
    for c in range(NCORE):
        oh[c, sarange % 128, (sarange // 128) * 128 + d_loc[c]] = w_slot[c]
    oh = oh.astype(ml_dtypes.bfloat16)

    idxw = idx16.reshape(NCORE, S // 16, 16).transpose(0, 2, 1)
    idxw = np.ascontiguousarray(np.tile(idxw, (1, 2, 1)))  # [NCORE,32,S/16]

    # gather segments: (pass, chunk-group of CGRP) -> (k0,k1,p,t0,ntiles)
    segs = []
    for p in range(2):
        for k0 in range(0, CH, CGRP):
            k1 = min(k0 + CGRP, CH)
            segs.append((k0, k1, p, int(toff[p, k0]),
                         int(tiles[k0:k1, p].sum())))
    return dict(tiles=tiles, toff=toff, TT=TT, S=S, oh=oh, idxw=idxw, segs=segs)


def _dig(d, ks):
    for k in ks:
        d = d[k]
    return d


def _prep_params(params):
    p = params
    g = lambda *ks: _np(_dig(p, ks)).astype(np.float32)
    s1 = 1.0 / np.sqrt(1.0 + BN_EPS)
    out = {}
    W1 = g("encoder", 0, "lin", "W") * (g("encoder", 0, "bn", "g") * s1)[None, :]
    b1 = (g("encoder", 0, "lin", "b") * (g("encoder", 0, "bn", "g") * s1)
          + g("encoder", 0, "bn", "b"))
    W1p = np.zeros((KPAD, 32), np.float32)
    W1p[:D_IN] = W1
    w1blob = np.zeros((128, KC * 32), np.float32)
    for kc in range(KC):
        w1blob[:, kc * 32:(kc + 1) * 32] = W1p[kc * 128:(kc + 1) * 128]
    out["w1"] = w1blob.astype(ml_dtypes.bfloat16)
    out["b1"] = np.tile(b1, 4)[:, None].astype(np.float32)          # [128,1]
    W2 = g("encoder", 1, "lin", "W") * (g("encoder", 1, "bn", "g") * s1)[None, :]
    b2 = (g("encoder", 1, "lin", "b") * (g("encoder", 1, "bn", "g") * s1)
          + g("encoder", 1, "bn", "b"))
    out["w2"] = np.tile(W2, (4, 1)).astype(np.float32)              # [128,20]
    out["b2"] = b2[None, :].astype(np.float32)                      # [1,20]
    out["w0"] = np.concatenate([g("trans", "fc0", "W"),
                                g("trans", "fc0", "b")[None, :]], 0)  # [21,64]
    out["ln0g"] = g("trans", "ln0", "g")[None, :]
    out["ln0b"] = g("trans", "ln0", "b")[None, :]
    for li in range(2):
        lp = p["trans"]["layers"][li]
        wq = np.concatenate([_np(lp["q"]["W"]), _np(lp["q"]["b"])[None, :]], 0)
        wk = np.concatenate([_np(lp["k"]["W"]), _np(lp["k"]["b"])[None, :]], 0)
        wv = np.concatenate([_np(lp["v"]["W"]), _np(lp["v"]["b"])[None, :]], 0)
        out[f"wqkv{li}"] = np.concatenate([wq, wk, wv], 1).astype(np.float32)
        out[f"ln{li + 1}g"] = _np(lp["ln"]["g"])[None, :].astype(np.float32)
        out[f"ln{li + 1}b"] = _np(lp["ln"]["b"])[None, :].astype(np.float32)
    sbn = 1.0 / np.sqrt(1.0 + BN1D_EPS)
    Wc1 = g("gcn", "conv1", "W") * (g("gcn", "bn", "g") * sbn)[None, :]
    bg1 = (g("gcn", "conv1", "b") * (g("gcn", "bn", "g") * sbn)
           + g("gcn", "bn", "b"))
    out["wc1"] = np.concatenate([Wc1, np.zeros((1, HID), np.float32)], 0)
    out["bg1"] = bg1[None, :]
    out["wc2"] = np.concatenate([g("gcn", "conv2", "W"),
                                 np.zeros((1, HID), np.float32)], 0)
    out["bc2"] = g("gcn", "conv2", "b")[None, :]
    out["wsg"] = np.concatenate([g("sg_fc", "W"), g("sg_fc", "b")[None, :]], 0)
    Wm, bm = g("conv_mean", "W"), g("conv_mean", "b")
    Wl, bl = g("conv_logvar", "W"), g("conv_logvar", "b")
    out["wmlv"] = np.concatenate(
        [np.concatenate([Wm, Wl], 1), np.concatenate([bm, bl])[None, :]], 0)
    Wd = g("decoder", 0, "lin", "W") * (g("decoder", 0, "bn", "g") * s1)[None, :]
    bd = (g("decoder", 0, "lin", "b") * (g("decoder", 0, "bn", "g") * s1)
          + g("decoder", 0, "bn", "b"))
    out["wd"] = np.concatenate([Wd, bd[None, :]], 0)                # [29,32]
    out["wout"] = np.concatenate([g("dec_out", "W"),
                                  g("dec_out", "b")[None, :]], 0)   # [33,3000]
    C = g("cluster")
    out["cq"] = np.concatenate([-2.0 * C.T, (C * C).sum(1)[None, :]], 0)
    for k, v in out.items():
        if v.dtype in (np.float32, np.float64):
            out[k] = np.ascontiguousarray(v, dtype=np.float32)
    return out


# --------------------------------------------------------------------------
# device program
# --------------------------------------------------------------------------
def _build_program(plan):
    nc = bacc.Bacc("TRN2", target_bir_lowering=False, debug=False,
                   num_devices=NCORE)
    S = plan["S"]
    x_d = nc.dram_tensor("x_in", [NPAD, KPAD], BF16, kind="ExternalInput")
    oh_d = nc.dram_tensor("oh_in", [128, S], BF16, kind="ExternalInput")
    idx_d = nc.dram_tensor("idx_in", [32, S // 16], I16, kind="ExternalInput")
    wshapes = dict(
        w1=[128, KC * 32], b1=[128, 1], w2=[128, 20], b2=[1, 20], w0=[21, 64],
        ln0g=[1, 64], ln0b=[1, 64], wqkv0=[65, 192], wqkv1=[65, 192],
        ln1g=[1, 64], ln1b=[1, 64], ln2g=[1, 64], ln2b=[1, 64],
        wc1=[21, 64], bg1=[1, 64], wc2=[65, 64], bc2=[1, 64], wsg=[65, 64],
        wmlv=[65, 16], wd=[29, 32], wout=[33, 3000], cq=[29, 20])
    wd_d = {k: nc.dram_tensor(k + "_in", shp, BF16 if k == "w1" else F32,
                              kind="ExternalInput")
            for k, shp in wshapes.items()}
    feat_out = nc.dram_tensor("feat_out", [128, CH * 20], F32, kind="ExternalOutput")
    mulv_out = nc.dram_tensor("mulv_out", [128, CH * 16], F32, kind="ExternalOutput")
    q_out = nc.dram_tensor("q_out", [128, CH * 20], F32, kind="ExternalOutput")
    de_out = nc.dram_tensor("de_out", [NPAD, 3000], F32, kind="ExternalOutput")

    with tile.TileContext(nc) as tc:
        _emit(nc, tc, plan, x_d, oh_d, idx_d, wd_d,
              feat_out, mulv_out, q_out, de_out)
    nc.compile()
    return nc


def _emit(nc, tc, plan, x_d, oh_d, idx_d, wd_d, feat_out, mulv_out, q_out, de_out):
    import os
    PH = os.environ.get("K_PHASES", "BACD")
    tiles, toff, TT, S, segs = (plan["tiles"], plan["toff"], plan["TT"],
                                plan["S"], plan["segs"])
    f32 = F32
    RG = [list(range(NCORE))]

    const = tc.alloc_tile_pool(name="const", bufs=1)
    dram = tc.alloc_tile_pool(name="dram", bufs=1, space="DRAM")
    feat_dram = dram.tile([128, CH * 20], F32, name="feat_dram")
    mulv_dram = dram.tile([128, CH * 16], F32, name="mulv_dram")

    # ---------------- constants ----------------
    w_sb = {}
    for k in wd_d:
        w_sb[k] = const.tile(list(wd_d[k].shape), wd_d[k].dtype, name=f"w_{k}")
        nc.sync.dma_start(w_sb[k][:], wd_d[k].ap())
    idx_sb = const.tile([32, S // 16], I16, name="idx_sb")
    nc.sync.dma_start(idx_sb[:], idx_d.ap())

    ident_i = const.tile([128, 128], I32, name="ident_i")
    nc.gpsimd.iota(ident_i[:], pattern=[[1, 128]], base=0, channel_multiplier=0)
    icol_i = const.tile([128, 1], I32, name="icol_i")
    nc.gpsimd.iota(icol_i[:], pattern=[[0, 1]], base=0, channel_multiplier=1)
    ident_f = const.tile([128, 128], F32, name="ident_f")
    nc.vector.tensor_copy(ident_f[:], ident_i[:])
    icol_f = const.tile([128, 1], F32, name="icol_f")
    nc.vector.tensor_copy(icol_f[:], icol_i[:])
    ident = const.tile([128, 128], F32, name="ident")
    nc.vector.tensor_scalar(ident[:], ident_f[:], icol_f[:], None, op0=OP.is_equal)
    ones_col = const.tile([128, 1], F32, name="ones_col")
    nc.vector.memset(ones_col[:], 1.0)
    eps_col = const.tile([128, 1], F32, name="eps_col")
    nc.vector.memset(eps_col[:], LN_EPS)
    ones_row = const.tile([1, NPAD], F32, name="ones_row")
    nc.vector.memset(ones_row[:], 1.0)
    zeros_pad = const.tile([32, 192], F32, name="zeros_pad")
    nc.vector.memset(zeros_pad[:], 0.0)

    def pbc(name, src, width):
        t = const.tile([128, width], F32, name=f"pb_{name}")
        nc.gpsimd.partition_broadcast(t[:], src[:])
        return t
    b2_r = pbc("b2", w_sb["b2"], 20)
    ln0g_r = pbc("ln0g", w_sb["ln0g"], 64)
    ln0b_r = pbc("ln0b", w_sb["ln0b"], 64)
    ln1g_r = pbc("ln1g", w_sb["ln1g"], 64)
    ln1b_r = pbc("ln1b", w_sb["ln1b"], 64)
    ln2g_r = pbc("ln2g", w_sb["ln2g"], 64)
    ln2b_r = pbc("ln2b", w_sb["ln2b"], 64)
    bg1_r = pbc("bg1", w_sb["bg1"], 64)
    bc2_r = pbc("bc2", w_sb["bc2"], 64)

    def bc3(t2d, nt, width):
        return t2d[:].unsqueeze(1).broadcast_to([128, nt, width])

    def bc_col(t2d, width):
        shp = list(t2d.shape)
        return t2d[:].unsqueeze(2).broadcast_to(shp + [width])

    def elu_chain(pool, out_ap, in_ap, P, Fr, bias_col=None, bias_row3=None,
                  row_w=None):
        m = pool.tile([P, Fr], f32, name="elu_m", tag="elu_m")
        pos = pool.tile([P, Fr], f32, name="elu_p", tag="elu_p")
        m_ap, p_ap = m[:], pos[:]
        if bias_row3 is not None:
            m_ap = m[:].rearrange("p (t f) -> p t f", f=row_w)
            p_ap = pos[:].rearrange("p (t f) -> p t f", f=row_w)
            nc.vector.tensor_tensor(m_ap, in_ap, bias_row3, op=OP.add)
            nc.vector.tensor_scalar(pos[:], m[:], 0.0, None, op0=OP.max)
            nc.vector.tensor_scalar(m[:], m[:], 0.0, None, op0=OP.min)
        elif bias_col is not None:
            nc.vector.tensor_scalar(m_ap, in_ap, bias_col, 0.0, op0=OP.add, op1=OP.min)
            nc.vector.tensor_scalar(p_ap, in_ap, bias_col, 0.0, op0=OP.add, op1=OP.max)
        else:
            nc.vector.tensor_scalar(m_ap, in_ap, 0.0, None, op0=OP.min)
            nc.vector.tensor_scalar(p_ap, in_ap, 0.0, None, op0=OP.max)
        nc.scalar.activation(m[:], m[:], ACT.Exp)
        nc.vector.tensor_tensor(pos[:], pos[:], m[:], op=OP.add)
        nc.vector.tensor_scalar(out_ap, p_ap, 1.0, None, op0=OP.subtract)

    def layer_norm(pool, out_t, in_t, g_r, b_r, relu=False):
        v3 = in_t[:].rearrange("p (t f) -> p t f", f=64)
        mu = pool.tile([128, CH], f32, name="ln_mu", tag="ln_mu")
        nc.vector.tensor_reduce(mu[:], v3, axis=AX.X, op=OP.add)
        nc.vector.tensor_scalar(mu[:], mu[:], 1.0 / 64, None, op0=OP.mult)
        xc = pool.tile([128, CH * 64], f32, name="ln_xc", tag="bigscr2")
        xc3 = xc[:].rearrange("p (t f) -> p t f", f=64)
        nc.vector.tensor_tensor(xc3, v3, bc_col(mu, 64), op=OP.subtract)
        sq = pool.tile([128, CH * 64], f32, name="ln_sq", tag="bigA")
        nc.vector.tensor_tensor(sq[:], xc[:], xc[:], op=OP.mult)
        var = pool.tile([128, CH], f32, name="ln_var", tag="ln_var")
        nc.vector.tensor_reduce(var[:], sq[:].rearrange("p (t f) -> p t f", f=64),
                                axis=AX.X, op=OP.add)
        sd = pool.tile([128, CH], f32, name="ln_sd", tag="ln_sd")
        nc.scalar.activation(sd[:], var[:], ACT.Sqrt, bias=eps_col[:], scale=1.0 / 64)
        nc.vector.reciprocal(sd[:], sd[:])
        o3 = out_t[:].rearrange("p (t f) -> p t f", f=64)
        nc.vector.tensor_tensor(o3, xc3, bc_col(sd, 64), op=OP.mult)
        nc.vector.tensor_tensor(o3, o3, bc3(g_r, CH, 64), op=OP.mult)
        nc.vector.tensor_tensor(o3, o3, bc3(b_r, CH, 64), op=OP.add)
        if relu:
            nc.vector.tensor_scalar(out_t[:], out_t[:], 0.0, None, op0=OP.max)

    def transp_into(psp, src_ap, dst_t, t, Fr):
        tp = psp.tile([Fr, 128], f32, name="tp_ps", tag="tp_ps")
        nc.tensor.transpose(tp[:], src_ap, ident[:])
        nc.any.tensor_copy(dst_t[0:Fr, t * 128:(t + 1) * 128], tp[:])

    # ================= Phase B: encoder =================
    ph_c = tc.alloc_tile_pool(name="ph_c", bufs=1)
    ph_b = tc.alloc_tile_pool(name="ph_b", bufs=1)
    o1 = ph_b.tile([128, 1664], f32, name="o1")
    with (tc.tile_pool(name="xtp", bufs=3) as xtp,
          tc.tile_pool(name="enc_ps", bufs=2, space="PSUM") as enc_ps,
          tc.tile_pool(name="enc_tmp", bufs=2) as enc_tmp):
        for gi, (g0, gsz) in enumerate(NODEGROUPS):
            ncb = max(1, gsz // 512)
            cbw = min(gsz, 512)
            P = 32 * ncb
            pb = enc_ps.tile([P, 512], f32, name="enc_pb", tag="enc_pb")
            for kc in range(KC):
                xt = xtp.tile([128, gsz], BF16, name="xt", tag="xt")
                nc.sync.dma_start(
                    xt[:], x_d.ap()[g0:g0 + gsz, kc * 128:(kc + 1) * 128],
                    transpose=True)
                for cb in range(ncb):
                    nc.tensor.matmul(
                        pb[32 * cb:32 * cb + 32, 0:cbw],
                        w_sb["w1"][:, kc * 32:(kc + 1) * 32],
                        xt[:, cb * 512:cb * 512 + cbw],
                        start=(kc == 0), stop=(kc == KC - 1),
                        tile_position=(0, 32 * cb))
            c0 = g0 // 4
            elu_chain(enc_tmp, o1[0:P, c0:c0 + cbw], pb[0:P, 0:cbw], P, cbw,
                      bias_col=w_sb["b1"][0:P, :])

    featT = ph_b.tile([21, NPAD], f32, name="featT")
    nc.sync.dma_start(featT[20:21, :], ones_row[:])
    feat_nm = ph_b.tile([128, CH * 20], f32, name="feat_nm")
    with (tc.tile_pool(name="l2_ps", bufs=4, space="PSUM") as l2_ps,
          tc.tile_pool(name="l2_tmp", bufs=2) as l2_tmp):
        for t in range(CH):
            j = t // 4
            pbase = 32 * (j % 4)
            cols = 512 * (j // 4) + 128 * (t % 4)
            fp = l2_ps.tile([128, 20], f32, name="fp", tag="fp")
            nc.tensor.matmul(fp[:], o1[pbase:pbase + 32, cols:cols + 128],
                             w_sb["w2"][pbase:pbase + 32, :],
                             start=True, stop=True, tile_position=(pbase, 0))
            nc.any.tensor_copy(feat_nm[:, t * 20:(t + 1) * 20], fp[:])
        f3 = feat_nm[:].rearrange("p (t f) -> p t f", f=20)
        elu_chain(l2_tmp, f3, f3, 128, CH * 20,
                  bias_row3=bc3(b2_r, CH, 20), row_w=20)
        nc.sync.dma_start(feat_out.ap(), feat_nm[:])
        nc.sync.dma_start(feat_dram[:], feat_nm[:])
    with tc.tile_pool(name="l2t_ps", bufs=2, space="PSUM") as l2t_ps:
        for t in range(CH):
            transp_into(l2t_ps, feat_nm[:, t * 20:(t + 1) * 20], featT, t, 20)

    if PH == "B":
        ph_b.release()
        ph_c.release()
        for p in (dram, const):
            p.release()
        return
    # ================= Phase C =================
    def hshard_and_ag(lhsT_t, w_t, name, ps_pool, st_pool):
        shard = dram.tile([NPAD, 64], f32, name=f"shard_{name}")
        tbl = dram.tile([TBL, 64], f32, name=f"tbl_{name}", addr_space="Shared")
        for t in range(CH):
            hp = ps_pool.tile([128, 64], f32, name=f"hp_{name}", tag="hp")
            nc.tensor.matmul(hp[:], lhsT_t[:, t * 128:(t + 1) * 128], w_t[:],
                             start=True, stop=True)
            hs = st_pool.tile([128, 64], f32, name=f"hs_{name}", tag="hs", bufs=4)
            nc.any.tensor_copy(hs[:], hp[:])
            nc.sync.dma_start(shard[:][t * 128:(t + 1) * 128, :], hs[:])
        nc.gpsimd.collective_compute("AllGather", OP.bypass, replica_groups=RG,
                                     ins=[shard[:]], outs=[tbl[:]])
        return tbl

    def conv_agg(tbl, agg_nm, gpool, g16pool, ohpool, agg_ps):
        nseg = len(segs) // 2
        for sg in range(nseg):
            bufs = {}
            for p in range(2):
                k0, k1, _, t0, nt = segs[p * nseg + sg]
                gb = gpool.tile([128, nt, 64], f32, name="gb", tag="gb")
                nc.gpsimd.dma_gather(
                    gb[:], tbl[:][p * SPLIT:(p + 1) * SPLIT, :],
                    idx_sb[:, t0 * 8:(t0 + nt) * 8],
                    num_idxs=nt * 128, num_idxs_reg=nt * 128, elem_size=64,
                    single_packet=False)
                gb16 = g16pool.tile([128, nt * 64], BF16, name="gb16", tag="gb16")
                nc.vector.tensor_copy(gb16[:], gb[:].rearrange("p t f -> p (t f)"))
                ohb = ohpool.tile([128, nt * 128], BF16, name="ohb", tag="ohb")
                nc.sync.dma_start(ohb[:], oh_d.ap()[:, t0 * 128:(t0 + nt) * 128])
                bufs[p] = (t0, nt, gb16, ohb)
            k0, k1 = segs[sg][0], segs[sg][1]
            for k in range(k0, k1):
                ap_ = agg_ps.tile([128, 64], f32, name="aggp", tag="aggp")
                nmm = int(tiles[k, 0] + tiles[k, 1])
                i = 0
                for p in range(2):
                    t0, nt, gb16, ohb = bufs[p]
                    for t in range(int(toff[p, k]), int(toff[p, k] + tiles[k, p])):
                        rel = t - t0
                        nc.tensor.matmul(
                            ap_[:], ohb[:, rel * 128:(rel + 1) * 128],
                            gb16[:, rel * 64:(rel + 1) * 64],
                            start=(i == 0), stop=(i == nmm - 1))
                        i += 1
                nc.any.tensor_copy(agg_nm[:, k * 64:(k + 1) * 64], ap_[:])

    # --- fc0 + LN + relu ---
    t_nm = ph_c.tile([128, CH * 64], f32, name="t_nm")
    with (tc.tile_pool(name="fc0_ps", bufs=2, space="PSUM") as fc0_ps,
          tc.tile_pool(name="att0_tmp", bufs=1) as att0_tmp):
        for t in range(CH):
            tp0 = fc0_ps.tile([128, 64], f32, name="tp0", tag="tp0")
            nc.tensor.matmul(tp0[:], featT[:, t * 128:(t + 1) * 128],
                             w_sb["w0"][:], start=True, stop=True)
            nc.any.tensor_copy(t_nm[:, t * 64:(t + 1) * 64], tp0[:])
        layer_norm(att0_tmp, t_nm, t_nm, ln0g_r, ln0b_r, relu=True)

    # --- h1 + AG(1) early ---
    with (tc.tile_pool(name="h1_ps", bufs=2, space="PSUM") as h1_ps,
          tc.tile_pool(name="h1_st", bufs=1) as h1_st):
        tbl1 = hshard_and_ag(featT, w_sb["wc1"], "h1", h1_ps, h1_st)

    ph_b.release()

    # --- conv1 aggregation emitted BEFORE attention so Q7 gather generation
    # overlaps attention compute (Tile schedules by deps) ---
    g_nm = ph_c.tile([128, CH * 64], f32, name="g_nm")
    gpool = tc.alloc_tile_pool(name="gb_pool", bufs=3)
    g16pool = tc.alloc_tile_pool(name="gb16_pool", bufs=3)
    ohpool = tc.alloc_tile_pool(name="oh_pool", bufs=3)
    agg_ps = tc.alloc_tile_pool(name="agg_ps", bufs=2, space="PSUM")
    conv_agg(tbl1, g_nm, gpool, g16pool, ohpool, agg_ps)
    g3 = g_nm[:].rearrange("p (t f) -> p t f", f=64)
    nc.vector.tensor_tensor(g3, g3, bc3(bg1_r, CH, 64), op=OP.add)
    nc.vector.tensor_scalar(g_nm[:], g_nm[:], 0.0, None, op0=OP.max)

    # --- attention layers ---
    x1 = ph_c.tile([128, CH * 64], f32, name="x1")
    n_f = float(N)
    if "G" in PH:
        nc.vector.tensor_copy(x1[:], t_nm[:])
    for li in ([] if "G" in PH else range(2)):
        lg = (ln1g_r, ln2g_r)[li]
        lb = (ln1b_r, ln2b_r)[li]
        with (tc.tile_pool(name=f"at{li}", bufs=1) as at,
              tc.tile_pool(name=f"at{li}_ps", bufs=2, space="PSUM") as atps,
              tc.tile_pool(name=f"at{li}_ps2", bufs=1, space="PSUM") as atps2):
            tT = at.tile([65, NPAD], f32, name="tT", tag="bigA")
            nc.sync.dma_start(tT[64:65, :], ones_row[:])
            for t in range(CH):
                transp_into(atps, t_nm[:, t * 64:(t + 1) * 64], tT, t, 64)
            qkv = at.tile([128, CH * 192], f32, name="qkv")
            for t in range(CH):
                qp = atps.tile([128, 192], f32, name="qp", tag="mm128")
                nc.tensor.matmul(qp[:], tT[:, t * 128:(t + 1) * 128],
                                 w_sb[f"wqkv{li}"][:], start=True, stop=True)
                nc.any.tensor_copy(qkv[:, t * 192:(t + 1) * 192], qp[:])
            nc.sync.dma_start(qkv[106:128, 48 * 192:49 * 192], zeros_pad[0:22, :])
            qv = qkv[:].rearrange("p (t c) -> p t c", c=192)
            kvs_ps = atps2.tile([64, 64], f32, name="kvs_ps", tag="kvs")
            ks_ps = atps2.tile([1, 66], f32, name="ks_ps", tag="ks")
            for t in range(CH):
                ksl = qkv[:, t * 192 + 64:t * 192 + 128]
                vsl = qkv[:, t * 192 + 128:t * 192 + 192]
                nc.tensor.matmul(kvs_ps[:], ksl, vsl,
                                 start=(t == 0), stop=(t == CH - 1))
                nc.tensor.matmul(ks_ps[:, 0:64], ones_col[:], ksl,
                                 start=(t == 0), stop=(t == CH - 1))
            sq = at.tile([128, CH * 64], f32, name="ssq_sq", tag="bigA")
            ssqc = at.tile([128, 2], f32, name="ssqc")
            for j, off in enumerate((0, 64)):
                nc.scalar.activation(
                    sq[:].rearrange("p (t f) -> p t f", f=64),
                    qv[:, :, off:off + 64], ACT.Square)
                nc.vector.tensor_reduce(ssqc[:, j:j + 1], sq[:], axis=AX.X,
                                        op=OP.add)
            ssq_ps = ks_ps[:, 64:66]
            nc.tensor.matmul(ssq_ps, ones_col[:], ssqc[:], start=True, stop=True,
                             skip_group_check=True)
            # stage pieces to DRAM individually (no cross-partition copies)
            kvs_sb = at.tile([64, 64], f32, name="kvs_sb")
            nc.any.tensor_copy(kvs_sb[:], kvs_ps[:])
            tail_sb = at.tile([1, 128], f32, name="tail_sb")
            nc.vector.memset(tail_sb[:], 0.0)
            nc.vector.tensor_copy(tail_sb[:, 0:64], ks_ps[:, 0:64])
            nc.vector.tensor_copy(tail_sb[:, 64:66], ssq_ps)
            ar_i = dram.tile([66, 64], f32, name=f"ar_i{li}")
            ar_o = dram.tile([66, 64], f32, name=f"ar_o{li}", addr_space="Shared")
            nc.sync.dma_start(ar_i[:][0:64, :], kvs_sb[:])
            nc.sync.dma_start(ar_i[:][64:66, :].rearrange("a b -> (a b)").unsqueeze(0),
                              tail_sb[:])
            nc.gpsimd.collective_compute("AllReduce", OP.add, replica_groups=RG,
                                         ins=[ar_i[:]], outs=[ar_o[:]])
            kvs_aug = at.tile([64, 65], f32, name="kvs_aug")
            nc.sync.dma_start(kvs_aug[:, 0:64], ar_o[:][0:64, :])
            nc.sync.dma_start(kvs_aug[:, 64:65],
                              ar_o[:][64:65, :].rearrange("o f -> f o"))
            ssq_sb = at.tile([1, 2], f32, name="ssq_sb")
            nc.sync.dma_start(ssq_sb[:], ar_o[:][65:66, 0:2])
            inv1 = at.tile([1, 1], f32, name="inv1")
            nc.vector.tensor_tensor(inv1[:], ssq_sb[:, 0:1], ssq_sb[:, 1:2],
                                    op=OP.mult)
            nc.scalar.activation(inv1[:], inv1[:], ACT.Sqrt)
            nc.vector.reciprocal(inv1[:], inv1[:])
            inv_bc = at.tile([128, 1], f32, name="inv_bc")
            nc.gpsimd.partition_broadcast(inv_bc[:], inv1[:])
            nc.vector.tensor_scalar(kvs_aug[:], kvs_aug[:], inv_bc[0:64, :],
                                    None, op0=OP.mult)
            num_nm = at.tile([128, CH * 65], f32, name="num_nm", tag="bigA")
            with tc.tile_pool(name=f"qt{li}", bufs=3) as qtp:
                for t in range(CH):
                    qT_ps = atps.tile([64, 128], f32, name="qT_ps", tag="tp_ps")
                    nc.tensor.transpose(qT_ps[:], qkv[:, t * 192:t * 192 + 64],
                                        ident[:])
                    qT = qtp.tile([64, 128], f32, name="qT", tag="qT")
                    nc.any.tensor_copy(qT[:], qT_ps[:])
                    np_ = atps.tile([128, 65], f32, name="np_", tag="mm128")
                    nc.tensor.matmul(np_[:], qT[:], kvs_aug[:],
                                     start=True, stop=True)
                    nc.any.tensor_copy(num_nm[:, t * 65:(t + 1) * 65], np_[:])
            nv = num_nm[:].rearrange("p (t c) -> p t c", c=65)
            den = at.tile([128, CH], f32, name="den")
            nc.vector.tensor_scalar(den[:], nv[:, :, 64:65].squeeze(2), n_f,
                                    None, op0=OP.add)
            nc.vector.reciprocal(den[:], den[:])
            scr = at.tile([128, CH * 64], f32, name="a_scr", tag="bigscr2")
            s3 = scr[:].rearrange("p (t f) -> p t f", f=64)
            nc.vector.tensor_scalar(s3, qv[:, :, 128:192], n_f, None, op0=OP.mult)
            nc.vector.tensor_tensor(s3, s3, nv[:, :, 0:64], op=OP.add)
            nc.vector.tensor_tensor(s3, s3, bc_col(den, 64), op=OP.mult)
            nc.vector.tensor_scalar(scr[:], scr[:], ALPHA_TRANS, None, op0=OP.mult)
            nc.vector.tensor_scalar(t_nm[:], t_nm[:], 1.0 - ALPHA_TRANS, None,
                                    op0=OP.mult)
            nc.vector.tensor_tensor(t_nm[:], t_nm[:], scr[:], op=OP.add)
            layer_norm(at, x1 if li == 1 else t_nm, t_nm, lg, lb)

    # --- GCN aggregation chain (conv2 onward; conv1 emitted above) ---
    with (tc.tile_pool(name="cv_tmp", bufs=1) as cv_tmp,
          tc.tile_pool(name="cv_ps", bufs=2, space="PSUM") as cv_ps):
        gT = cv_tmp.tile([65, NPAD], f32, name="gT", tag="convT")
        nc.sync.dma_start(gT[64:65, :], ones_row[:])
        for t in range(CH):
            transp_into(cv_ps, g_nm[:, t * 64:(t + 1) * 64], gT, t, 64)
        tbl2 = hshard_and_ag(gT, w_sb["wc2"], "h2", cv_ps, cv_tmp)
        x2_nm = cv_tmp.tile([128, CH * 64], f32, name="x2_nm", tag="convA")
        conv_agg(tbl2, x2_nm, gpool, g16pool, ohpool, agg_ps)
        x23 = x2_nm[:].rearrange("p (t f) -> p t f", f=64)
        nc.vector.tensor_tensor(x23, x23, bc3(bc2_r, CH, 64), op=OP.add)
        nc.vector.tensor_scalar(x2_nm[:], x2_nm[:], GRAPH_W, None, op0=OP.mult)
        nc.vector.tensor_scalar(x1[:], x1[:], 1.0 - GRAPH_W, None, op0=OP.mult)
        nc.vector.tensor_tensor(x2_nm[:], x2_nm[:], x1[:], op=OP.add)
        cxT = cv_tmp.tile([65, NPAD], f32, name="cxT", tag="convT")
        nc.sync.dma_start(cxT[64:65, :], ones_row[:])
        for t in range(CH):
            transp_into(cv_ps, x2_nm[:, t * 64:(t + 1) * 64], cxT, t, 64)
        tbl3 = hshard_and_ag(cxT, w_sb["wsg"], "cx", cv_ps, cv_tmp)
        aggcx = cv_tmp.tile([128, CH * 64], f32, name="aggcx", tag="convA")
        mulv_nm = cv_tmp.tile([128, CH * 16], f32, name="mulv_nm")
        conv_agg(tbl3, aggcx, gpool, g16pool, ohpool, agg_ps)
        acxT = cv_tmp.tile([65, NPAD], f32, name="acxT", tag="convT")
        nc.sync.dma_start(acxT[64:65, :], ones_row[:])
        for t in range(CH):
            transp_into(cv_ps, aggcx[:, t * 64:(t + 1) * 64], acxT, t, 64)
        for t in range(CH):
            mp = cv_ps.tile([128, 16], f32, name="mp", tag="hp")
            nc.tensor.matmul(mp[:], acxT[:, t * 128:(t + 1) * 128],
                             w_sb["wmlv"][:], start=True, stop=True)
            nc.any.tensor_copy(mulv_nm[:, t * 16:(t + 1) * 16], mp[:])
        nc.sync.dma_start(mulv_out.ap(), mulv_nm[:])
        nc.sync.dma_start(mulv_dram[:], mulv_nm[:])

    for _p in (agg_ps, ohpool, g16pool, gpool):
        _p.release()
    ph_c.release()
    if PH.endswith("C"):
        for p in (dram, const):
            p.release()
        return

    # ================= Phase D: z, q, decoder =================
    with (tc.tile_pool(name="ph_d", bufs=1) as ph_d,
          tc.tile_pool(name="d_tmp", bufs=2) as d_tmp):
        z_nm = ph_d.tile([128, CH * 28], f32, name="z_nm")
        z3 = z_nm[:].rearrange("p (t f) -> p t f", f=28)
        nc.sync.dma_start(z3[:, :, 0:20],
                          feat_dram[:].rearrange("p (t f) -> p t f", f=20))
        nc.sync.dma_start(
            z3[:, :, 20:28],
            mulv_dram[:].rearrange("p (t f) -> p t f", f=16)[:, :, 0:8])
        zT = ph_d.tile([29, NPAD], f32, name="zT", tag="dzT")
        nc.sync.dma_start(zT[28:29, :], ones_row[:])
        with tc.tile_pool(name="zt_ps", bufs=2, space="PSUM") as zt_ps:
            for t in range(CH):
                transp_into(zt_ps, z_nm[:, t * 28:(t + 1) * 28], zT, t, 28)
        zsq = ph_d.tile([128, CH * 28], f32, name="zsq")
        nc.scalar.activation(zsq[:], z_nm[:], ACT.Square)
        zn2 = ph_d.tile([128, CH], f32, name="zn2")
        nc.vector.tensor_reduce(zn2[:], zsq[:].rearrange("p (t f) -> p t f", f=28),
                                axis=AX.X, op=OP.add)
        q_nm = ph_d.tile([128, CH * 20], f32, name="q_nm")
        d_nm = ph_d.tile([128, CH * 32], f32, name="d_nm")
        with tc.tile_pool(name="qd_ps", bufs=2, space="PSUM") as qd_ps:
            for t in range(CH):
                qp_ = qd_ps.tile([128, 20], f32, name="qp_", tag="qp_")
                nc.tensor.matmul(qp_[:], zT[:, t * 128:(t + 1) * 128],
                                 w_sb["cq"][:], start=True, stop=True)
                nc.any.tensor_copy(q_nm[:, t * 20:(t + 1) * 20], qp_[:])
                dp = qd_ps.tile([128, 32], f32, name="dp", tag="dp")
                nc.tensor.matmul(dp[:], zT[:, t * 128:(t + 1) * 128],
                                 w_sb["wd"][:], start=True, stop=True)
                nc.any.tensor_copy(d_nm[:, t * 32:(t + 1) * 32], dp[:])
        q3 = q_nm[:].rearrange("p (t f) -> p t f", f=20)
        nc.vector.tensor_tensor(q3, q3, bc_col(zn2, 20), op=OP.add)
        nc.scalar.activation(q_nm[:], q_nm[:], ACT.Ln, bias=1.0,
                             scale=1.0 / ALPHA_DEC)
        nc.scalar.activation(q_nm[:], q_nm[:], ACT.Exp,
                             scale=-(ALPHA_DEC + 1.0) / 2.0)
        qs = ph_d.tile([128, CH], f32, name="qs")
        nc.vector.tensor_reduce(qs[:], q3, axis=AX.X, op=OP.add)
        nc.vector.reciprocal(qs[:], qs[:])
        nc.vector.tensor_tensor(q3, q3, bc_col(qs, 20), op=OP.mult)
        nc.sync.dma_start(q_out.ap(), q_nm[:])
        elu_chain(d_tmp, d_nm[:], d_nm[:], 128, CH * 32)
        dT = ph_d.tile([33, NPAD], f32, name="dT", tag="dzT")
        nc.sync.dma_start(dT[32:33, :], ones_row[:])
        with tc.tile_pool(name="dt_ps", bufs=2, space="PSUM") as dt_ps:
            for t in range(CH):
                transp_into(dt_ps, d_nm[:, t * 32:(t + 1) * 32], dT, t, 32)
        with (tc.tile_pool(name="de_ps", bufs=6, space="PSUM") as de_ps,
              tc.tile_pool(name="de_st", bufs=3) as de_st):
            for t in range(CH):
                stg_ = de_st.tile([128, 3000], f32, name="de_stage", tag="de_stage")
                for cb in range(6):
                    dep = de_ps.tile([128, 500], f32, name="dep", tag="dep")
                    nc.tensor.matmul(dep[:], dT[:, t * 128:(t + 1) * 128],
                                     w_sb["wout"][:, cb * 500:(cb + 1) * 500],
                                     start=True, stop=True)
                    nc.any.tensor_copy(stg_[:, cb * 500:(cb + 1) * 500], dep[:])
                nc.sync.dma_start(de_out.ap()[t * 128:(t + 1) * 128, :], stg_[:])

    for p in (dram, const):
        p.release()


# --------------------------------------------------------------------------
# public entry
# --------------------------------------------------------------------------
def kernel(x, params, edge_index):
    x = _np(x)
    ei = _np(edge_index)
    plan = _build_plan(ei)
    w = _prep_params(params)

    import os
    key = ("v1", os.environ.get("K_PHASES", "BACD"), plan["TT"], tuple(plan["tiles"].ravel().tolist()))
    if key not in _CACHE:
        _CACHE[key] = _build_program(plan)
    nc = _CACHE[key]

    xb = x.astype(ml_dtypes.bfloat16)
    base = {k + "_in": v for k, v in w.items()}
    in_maps = []
    for c in range(NCORE):
        xc = np.zeros((NPAD, KPAD), ml_dtypes.bfloat16)
        xc[:NPC, :D_IN] = xb[c * NPC:(c + 1) * NPC]
        m = dict(base)
        m["x_in"] = xc
        m["oh_in"] = np.ascontiguousarray(plan["oh"][c])
        m["idx_in"] = np.ascontiguousarray(plan["idxw"][c])
        in_maps.append(m)

    res = run_bass_kernel_spmd(nc, in_maps, core_ids=list(range(NCORE)))
    kernel.last_results = res

    def unpack(name, F):
        outs = []
        for c in range(NCORE):
            a = res.results[c][name]
            outs.append(a.reshape(128, CH, F).transpose(1, 0, 2).reshape(NPAD, F)[:NPC])
        return np.concatenate(outs, 0)

    feat_x = unpack("feat_out", 20)
    mulv = unpack("mulv_out", 16)
    q = unpack("q_out", 20)
    mu = np.ascontiguousarray(mulv[:, 0:8])
    logvar = np.ascontiguousarray(mulv[:, 8:16])
    de_feat = np.concatenate(
        [res.results[c]["de_out"][:NPC] for c in range(NCORE)], 0)
    z = np.concatenate([feat_x, mu], 1)
    return (z, mu, logvar, de_feat, q, feat_x, mu)
